# revision 34
# baseline (speedup 1.0000x reference)
"""Trainium2 Bass kernel for nn_Critic (two ragged GRUs + two MLP heads).

Sharding (8 NeuronCores):
  - cores 0-3: GRU1 + MLP head 1;  cores 4-7: GRU2 + MLP head 2
  - batch (2048) sorted by length descending; ranks 0:1024 are dealt
    round-robin to the 4 cores' chain A, ranks 1024:2048 to chain B.
    Chain B dies entirely once the ~median-length sequences finish.
  - two independent 256-wide recurrence chains per core hide each
    other's serial gate latency
  - staircase: at step t only the first m_t (still-running, sorted)
    columns of each chain are processed; a per-(t,col) "dead" row folded
    into the z-gate matmul (+LARGE -> z=1 -> h frozen) makes the even
    round-up exact.
  - [feature, batch] layout everywhere (h stored transposed): no
    transposes; input projection, biases, dead-mask and bhh_n are folded
    into the recurrent PSUM accumulation via extra K rows (fp16 matmuls,
    fp32 PSUM).
"""

import os
import sys
import numpy as np

if "/opt/trn_rl_repo" not in sys.path:
    sys.path.insert(0, "/opt/trn_rl_repo")

B, T, D, A, H = 2048, 64, 15, 1, 256
G3 = 3 * H  # 768
NGROUP = 4          # batch groups (cores per head)
BL = B // NGROUP    # 512 local sequences per core
CH = BL // 2        # 256 columns per chain
LARGE = 40.0        # z preactivation offset for dead columns -> z == 1

LAST_RESULTS = None  # BassKernelResults of the most recent run (for test.py)


def _mwidth(n):
    """chain width for n globally-active rows (symmetric deal, 8 chains)"""
    if n <= 0:
        return 0
    mt = (n + 7) // 8           # >= per-chain active count for any core
    mt += mt & 1                # round up to even
    return min(CH, mt)


def _build_schedule(lengths_sorted):
    """per-step m (same for both chains), identical across cores (SPMD)."""
    return [_mwidth(int((lengths_sorted > t).sum())) for t in range(T)]


def _build_program(m_sched):
    import concourse.mybir as mybir
    import concourse.tile as tile
    from concourse import bacc
    from concourse.tile import add_dep_helper

    f32 = mybir.dt.float32
    f16 = mybir.dt.float16
    AF = mybir.ActivationFunctionType
    OP = mybir.AluOpType

    nc = bacc.Bacc("TRN2", target_bir_lowering=False, debug=False, num_devices=8)

    sx_d = nc.declare_dram_parameter("sx", [128, T // 4, BL], f16, isOutput=False)
    whh_d = nc.declare_dram_parameter("whh", [128, 2, G3], f16, isOutput=False)
    wih_d = nc.declare_dram_parameter("wih", [128, 4, G3], f16, isOutput=False)
    wbhhn_d = nc.declare_dram_parameter("wbhhn", [128, 4, H], f16, isOutput=False)
    w1a_d = nc.declare_dram_parameter("w1a", [128, 2, 1024], f16, isOutput=False)
    w1b_d = nc.declare_dram_parameter("w1b", [16, 1024], f16, isOutput=False)
    w2_d = nc.declare_dram_parameter("w2", [128, 8, 1024], f16, isOutput=False)
    w3_d = nc.declare_dram_parameter("w3", [128, 8, 512], f16, isOutput=False)
    w4_d = nc.declare_dram_parameter("w4", [128, 4, 256], f16, isOutput=False)
    wq_d = nc.declare_dram_parameter("wq", [128, 2, 1], f16, isOutput=False)
    b1_d = nc.declare_dram_parameter("b1", [128, 8], f32, isOutput=False)
    b2_d = nc.declare_dram_parameter("b2", [128, 8], f32, isOutput=False)
    b3_d = nc.declare_dram_parameter("b3", [128, 4], f32, isOutput=False)
    b4_d = nc.declare_dram_parameter("b4", [128, 2], f32, isOutput=False)
    bq_d = nc.declare_dram_parameter("bq", [1, 1], f32, isOutput=False)
    aug_d = nc.declare_dram_parameter("aug", [16, BL], f16, isOutput=False)
    out_d = nc.declare_dram_parameter("out", [1, BL], f32, isOutput=True)

    with tile.TileContext(nc) as tc:
        with (
            tc.tile_pool(name="const", bufs=1) as cpool,
            tc.tile_pool(name="work", bufs=2) as wpool,
        ):
            # ---- resident tensors -------------------------------------
            sx_t = cpool.tile([128, T // 4, BL], f16, name="sx", tag="sx")
            for lo, hi in ((0, 1), (1, 2), (2, 4), (4, 8), (8, 16)):
                nc.sync.dma_start(out=sx_t[:, lo:hi, :], in_=sx_d[:, lo:hi, :])

            def load(shape, tag, src, dt=f16):
                t_ = cpool.tile(shape, dt, name=tag, tag=tag)
                nc.sync.dma_start(out=t_[:], in_=src[:])
                return t_

            whh_t = load([128, 2, G3], "whh", whh_d)
            wih_t = load([128, 4, G3], "wih", wih_d)
            wbhhn_t = load([128, 4, H], "wbhhn", wbhhn_d)
            w1a_t = load([128, 2, 1024], "w1a", w1a_d)
            w1b_t = load([16, 1024], "w1b", w1b_d)
            w2_t = load([128, 8, 1024], "w2", w2_d)
            w3_t = load([128, 8, 512], "w3", w3_d)
            w4_t = load([128, 4, 256], "w4", w4_d)
            wq_t = load([128, 2, 1], "wq", wq_d)
            b1_t = load([128, 8], "b1", b1_d, dt=f32)
            b2_t = load([128, 8], "b2", b2_d, dt=f32)
            b3_t = load([128, 4], "b3", b3_d, dt=f32)
            b4_t = load([128, 2], "b4", b4_d, dt=f32)
            bq_t = load([1, 1], "bq", bq_d, dt=f32)
            aug_t = load([16, BL], "aug", aug_d)

            # h chunks: 0,1 = chain A halves; 2,3 = chain B halves
            h_t = cpool.tile([128, 4, CH], f16, name="h", tag="h")
            nc.vector.memset(h_t[:], 0.0)

            # ---- GRU steps --------------------------------------------
            gpsum = []
            step_psum = {}

            def gru_step(q, t, m, small, part):
                strip = (t % 4) * 32
                blk = t // 4
                o = q * CH
                cw = 128 if small else CH
                pool = gpsum[1] if small else gpsum[0]
                rz = pool.tile([128, 4, cw], f32, name="rz",
                               tag=f"rz{q}{'s' if small else ''}")
                hnin = pool.tile([128, 4, cw], f32, name="hnin",
                                 tag=f"hnin{q}{'s' if small else ''}")
                sp = t % 4
                rhs_s = sx_t[:, blk, o : o + m]
                rhs_h = (h_t[:, 2 * q, 0:m], h_t[:, 2 * q + 1, 0:m])

                def emit_bank(tile_ap, flat):
                    insts = []
                    for i, (c, lhsT, rhs, tpos) in enumerate(flat):
                        inst = nc.tensor.matmul(
                            tile_ap[:, c, 0:m], lhsT, rhs,
                            start=(i == 0), stop=(i == len(flat) - 1),
                            tile_position=tpos)
                        if insts:
                            add_dep_helper(inst.ins, insts[-1].ins, sync=False,
                                           reason="psum group order")
                        insts.append(inst)

                def wih_mm(c, gcol):
                    return (c, wih_t[:, sp, gcol : gcol + 128], rhs_s, None)

                def wbh_mm(c, half):
                    return (c, wbhhn_t[:, sp, half * 128 : half * 128 + 128],
                            rhs_s, None)

                def whh_mm(c, k, gcol):
                    return (c, whh_t[:, k, gcol : gcol + 128], rhs_h[k], None)

                def whhs(c, gcol):
                    return [whh_mm(c, 0, gcol), whh_mm(c, 1, gcol)] if t > 0 else []

                # in bank first (h-independent), then r, hn, z
                if part == 0:
                    emit_bank(hnin, [wih_mm(2, 2 * H), wih_mm(3, 2 * H + 128)])
                    emit_bank(rz, [wih_mm(0, 0), wih_mm(1, 128)]
                              + whhs(0, 0) + whhs(1, 128))
                    emit_bank(hnin, [wbh_mm(0, 0), wbh_mm(1, 1)]
                              + whhs(0, 2 * H) + whhs(1, 2 * H + 128))
                    emit_bank(rz, [wih_mm(2, H), wih_mm(3, H + 128)]
                              + whhs(2, H) + whhs(3, H + 128))
                    step_psum[q] = (rz, hnin)
                    return
                rz, hnin = step_psum[q]

                rz_sb = wpool.tile([128, 4, CH], f16, name="rz_sb", tag=f"rz_sb{q}")
                nc.scalar.activation(rz_sb[:, 0:2, 0:m], rz[:, 0:2, 0:m],
                                     AF.Sigmoid)
                t1 = wpool.tile([128, 2, CH], f16, name="t1", tag=f"t1{q}")
                nc.vector.tensor_tensor(t1[:, :, 0:m], rz_sb[:, 0:2, 0:m],
                                        hnin[:, 0:2, 0:m], OP.mult)
                a_sb = wpool.tile([128, 2, CH], f16, name="a_sb", tag=f"a_sb{q}")
                nc.vector.tensor_tensor(a_sb[:, :, 0:m], t1[:, :, 0:m],
                                        hnin[:, 2:4, 0:m], OP.add)
                n_sb = wpool.tile([128, 2, CH], f16, name="n_sb", tag=f"n_sb{q}")
                nc.scalar.activation(n_sb[:, :, 0:m], a_sb[:, :, 0:m], AF.Tanh)
                nc.scalar.activation(rz_sb[:, 2:4, 0:m], rz[:, 2:4, 0:m],
                                     AF.Sigmoid)
                d_sb = wpool.tile([128, 2, CH], f16, name="d_sb", tag=f"d_sb{q}")
                zd = wpool.tile([128, 2, CH], f16, name="zd", tag=f"zd{q}")
                for half in (0, 1):  # per half: K-half mms can start earlier
                    nc.vector.tensor_tensor(d_sb[:, half, 0:m],
                                            h_t[:, 2 * q + half, 0:m],
                                            n_sb[:, half, 0:m], OP.subtract)
                    nc.vector.tensor_tensor(zd[:, half, 0:m],
                                            rz_sb[:, 2 + half, 0:m],
                                            d_sb[:, half, 0:m], OP.mult)
                    nc.vector.tensor_tensor(h_t[:, 2 * q + half, 0:m],
                                            n_sb[:, half, 0:m],
                                            zd[:, half, 0:m], OP.add)

            # once m <= 128 the back columns are done: shrink psum tiles
            # to 1 bank each and run the MLP on finished columns in parallel
            t_small = next((t for t in range(T)
                            if 0 < m_sched[t] <= 128), T)

            def mlp_phase(p, pool):
                # p=0: cols 0:128 of each chain; p=1: cols 128:256
                c0 = p * 128

                def rhs_h(k):
                    return h_t[:, k : k + 3 : 2, c0 : c0 + 128]

                def cols(src_ap):  # [*, {A c0:c0+128, B CH+c0:...}]
                    return src_ap[:, c0 : c0 + CH + 128 : CH][:, :, 0:128] \
                        if False else None

                def dense(n_mt, n_k, w_t, rhs_fn, b_t, out_shape, tag):
                    out_t = cpool.tile(out_shape, f16, name=tag, tag=tag)
                    for mt in range(n_mt):
                        ps = pool.tile([128, 2, 128], f32, name="mlp_ps",
                                       tag=f"mlp{p}")
                        for k in range(n_k):
                            nc.tensor.matmul(
                                ps[:], w_t[:, k, 128 * mt : 128 * (mt + 1)],
                                rhs_fn(k), start=(k == 0), stop=(k == n_k - 1))
                        nc.scalar.activation(out_t[:, mt, :, :], ps[:], AF.Relu,
                                             bias=b_t[:, mt : mt + 1])
                    return out_t

                aug_rhs = aug_t.rearrange("k (ch c) -> k ch c", ch=2)[
                    :, :, c0 : c0 + 128]
                a1_t = cpool.tile([128, 8, 2, 128], f16, name=f"a1_{p}",
                                  tag=f"a1_{p}")
                for mt in range(8):
                    ps = pool.tile([128, 2, 128], f32, name="mlp_ps",
                                   tag=f"mlp{p}")
                    nc.tensor.matmul(ps[:], w1b_t[:, 128 * mt : 128 * (mt + 1)],
                                     aug_rhs, start=True, stop=False)
                    for k in (0, 1):
                        nc.tensor.matmul(
                            ps[:], w1a_t[:, k, 128 * mt : 128 * (mt + 1)],
                            rhs_h(k), start=False, stop=(k == 1))
                    nc.scalar.activation(a1_t[:, mt, :, :], ps[:], AF.Relu,
                                         bias=b1_t[:, mt : mt + 1])

                a2_t = dense(8, 8, w2_t, lambda k: a1_t[:, k, :, :], b2_t,
                             [128, 8, 2, 128], f"a2_{p}")
                a3_t = dense(4, 8, w3_t, lambda k: a2_t[:, k, :, :], b3_t,
                             [128, 4, 2, 128], f"a3_{p}")
                a4_t = dense(2, 4, w4_t, lambda k: a3_t[:, k, :, :], b4_t,
                             [128, 2, 2, 128], f"a4_{p}")

                psq = pool.tile([1, 2, 128], f32, name="psq", tag=f"mlpq{p}",
                                bufs=1)
                for k in range(2):
                    nc.tensor.matmul(psq[:], wq_t[:, k, :], a4_t[:, k, :, :],
                                     start=(k == 0), stop=(k == 1))
                q_out = q_sb.rearrange("one (ch c) -> one ch c", ch=2)[
                    :, :, c0 : c0 + 128]
                nc.scalar.activation(q_out, psq[:], AF.Identity,
                                     bias=bq_t[0:1, 0:1])

            q_sb = cpool.tile([1, BL], f32, name="qsb", tag="qsb")
            with tc.tile_pool(name="psumBig", bufs=1, space="PSUM") as gpBig:
                gpsum.append(gpBig)
                for t in range(t_small):
                    if m_sched[t] > 0:
                        for part in (0, 1):
                            gru_step(0, t, m_sched[t], False, part)
                            gru_step(1, t, m_sched[t], False, part)
            with tc.tile_pool(name="psumSmall", bufs=1, space="PSUM") as gpSm:
                gpsum.append(gpSm)
                with tc.tile_pool(name="psumM1", bufs=2, space="PSUM") as mp1:
                    for t in range(t_small, T):
                        if m_sched[t] > 0:
                            for part in (0, 1):
                                gru_step(0, t, m_sched[t], True, part)
                                gru_step(1, t, m_sched[t], True, part)
                    mlp_phase(1, mp1)
            with tc.tile_pool(name="psumM0", bufs=4, space="PSUM") as mp0:
                mlp_phase(0, mp0)
            nc.sync.dma_start(out=out_d[:], in_=q_sb[:])

    nc.compile()
    return nc


def _host_prep(inputs, order):
    """Build per-core input maps. order: global indices sorted by length desc."""
    state = np.ascontiguousarray(inputs["state"], dtype=np.float32)
    action = np.ascontiguousarray(inputs["action"], dtype=np.float32)
    lengths = np.asarray(inputs["lengths"]).astype(np.int64)

    in_maps = []
    col_orders = []  # per core: global seq index per output column
    for core in range(8):
        g = core // 4 + 1
        bg = core % 4
        grp = order[bg::NGROUP]                    # 512, descending lengths
        chainA = grp[0::2]
        chainB = grp[1::2]
        seqs_mlp = np.concatenate([chainA, chainB])
        col_orders.append(seqs_mlp)

        sx = np.zeros((128, T // 4, BL), np.float16)
        for q, seqs in ((0, chainA), (1, chainB)):
            st = state[seqs]                                       # [256, T, 15]
            dead = (lengths[seqs][None, :] <= np.arange(T)[:, None])  # [T, 256]
            o = q * CH
            for r in range(4):
                ts_ = np.arange(r, T, 4)
                sx[r * 32 : r * 32 + 15, :, o : o + CH] = \
                    st[:, ts_, :].transpose(2, 1, 0)
                sx[r * 32 + 15, :, o : o + CH] = 1.0
                sx[r * 32 + 16, :, o : o + CH] = dead[ts_, :].astype(np.float16)

        wih = np.asarray(inputs[f"g{g}_Wih"], np.float32)   # [768, 15]
        whh = np.asarray(inputs[f"g{g}_Whh"], np.float32)   # [768, 256]
        bih = np.asarray(inputs[f"g{g}_bih"], np.float32)
        bhh = np.asarray(inputs[f"g{g}_bhh"], np.float32)

        whh_l = np.stack([whh[:, k * 128 : (k + 1) * 128].T
                          for k in (0, 1)], axis=1)          # [128, 2, 768]

        row_bias = bih + bhh
        row_bias[2 * H :] = bih[2 * H :]                     # n-block: bih only
        row_mask = np.zeros(G3, np.float32)
        row_mask[H : 2 * H] = LARGE
        wih_ext = np.zeros((128, 4, G3), np.float32)
        wbhhn = np.zeros((128, 4, H), np.float32)
        for r in range(4):  # strip r: nonzero rows only at 32r..32r+16
            wih_ext[r * 32 : r * 32 + 15, r, :] = wih.T
            wih_ext[r * 32 + 15, r, :] = row_bias
            wih_ext[r * 32 + 16, r, :] = row_mask
            wbhhn[r * 32 + 15, r, :] = bhh[2 * H :]

        def ktiles(w, nk):
            return np.stack([w[:, k * 128 : (k + 1) * 128].T
                             for k in range(nk)], axis=1)

        w1 = np.asarray(inputs[f"fc1_{g}_w"], np.float32)   # [1024, 272]
        w1perm = np.concatenate([w1[:, 16:272], w1[:, 0:16]], axis=1)
        w1a = ktiles(w1perm, 2)                             # [128, 2, 1024]
        w1b = np.ascontiguousarray(w1perm[:, 256:272].T)    # [16, 1024]
        w2l = ktiles(np.asarray(inputs[f"fc2_{g}_w"], np.float32), 8)
        w3l = ktiles(np.asarray(inputs[f"fc3_{g}_w"], np.float32), 8)
        w4l = ktiles(np.asarray(inputs[f"fc4_{g}_w"], np.float32), 4)
        wql = ktiles(np.asarray(inputs[f"q_{g}_w"], np.float32), 2)  # [128,2,1]

        def btile(name, nmt):
            return np.ascontiguousarray(
                np.asarray(inputs[name], np.float32).reshape(nmt, 128).T)

        aug = np.zeros((16, BL), np.float16)
        aug[0:15, :] = state[seqs_mlp, 0, :].T
        aug[15, :] = action[seqs_mlp, 0]

        f16c = lambda a: np.ascontiguousarray(a, dtype=np.float16)
        in_maps.append({
            "sx": np.ascontiguousarray(sx), "whh": f16c(whh_l),
            "wih": f16c(wih_ext), "wbhhn": f16c(wbhhn),
            "w1a": f16c(w1a), "w1b": f16c(w1b),
            "w2": f16c(w2l), "w3": f16c(w3l),
            "w4": f16c(w4l), "wq": f16c(wql),
            "b1": btile(f"fc1_{g}_b", 8), "b2": btile(f"fc2_{g}_b", 8),
            "b3": btile(f"fc3_{g}_b", 4), "b4": btile(f"fc4_{g}_b", 2),
            "bq": np.asarray(inputs[f"q_{g}_b"], np.float32).reshape(1, 1),
            "aug": aug,
        })
    return in_maps, col_orders


def kernel(**inputs):
    global LAST_RESULTS
    from concourse.bass_utils import run_bass_kernel_spmd

    lengths = np.asarray(inputs["lengths"]).astype(np.int64)
    order = np.argsort(-lengths, kind="stable")
    m_sched = _build_schedule(lengths[order])

    nc = _build_program(m_sched)
    in_maps, col_orders = _host_prep(inputs, order)

    trace = bool(int(os.environ.get("KERNEL_TRACE", "0")))
    res = run_bass_kernel_spmd(nc, in_maps, list(range(8)), trace=trace)
    LAST_RESULTS = res

    out1 = np.zeros((B, 1), np.float32)
    out2 = np.zeros((B, 1), np.float32)
    for core in range(8):
        vals = np.asarray(res.results[core]["out"]).reshape(BL)
        dst = out1 if core < 4 else out2
        dst[col_orders[core], 0] = vals
    return out1, out2


if __name__ == "__main__":
    sys.path.insert(0, os.path.dirname(os.path.abspath(__file__)))
    import reference

    inputs = {k: np.asarray(v) for k, v in reference.setup_inputs().items()}
    o1, o2 = kernel(**inputs)
    print("out1", o1[:4, 0])
    print("out2", o2[:4, 0])
